# revision 12
# baseline (speedup 1.0000x reference)
"""Trainium2 Bass kernel for BlockAttnResLayer.

Computation (reference):
  V = concat([blocks, partial[None]])            # [9, B*T, D]
  rms = sqrt(mean(V^2, -1) + 1e-8)
  logits[n,t] = (V[n,t,:] . (norm_scale*proj_w)) / rms[n,t]
  alpha = softmax(logits, axis=n)
  h = sum_n alpha * V
  f = gelu(h @ W1) @ W2                          # tanh-approx gelu
  new_partial = partial + f
  returns (h, new_partial)

Sharding: pure data-parallel over tokens (B*T = 4096 -> 512/core on 8 cores).
Weights replicated; FFN matmuls run in float32r (fp32 with 11-bit mantissa,
1 cycle/row on the PE at N>=256 vs 4 cycles/row for plain fp32).
"""
import numpy as np
from contextlib import ExitStack

import concourse.bass as bass
import concourse.bacc as bacc
import concourse.tile as tile
from concourse import mybir
from concourse.bass_utils import run_bass_kernel_spmd
from concourse.masks import make_identity

f32 = mybir.dt.float32
f32r = mybir.dt.float32r
AF = mybir.ActivationFunctionType
ALU = mybir.AluOpType

N_CORES = 8
NB = 8            # completed blocks
N1 = 9            # blocks + partial
B, T, D, F = 2, 2048, 2048, 8192
TOK = B * T       # 4096
TPC = TOK // N_CORES  # 512 tokens per core
P = 128
TT = TPC // P     # 4 token tiles per core
DC = D // P       # 16 d-chunks
FC = F // P       # 64 f-chunks
FH = FC // 2      # 32 f-chunks per half
NQ = D // 512     # 4 output column quarters
EPS = 1e-8


def round_f32r(x: np.ndarray) -> np.ndarray:
    """RNE-round fp32 to 11 explicit mantissa bits (the PE's fp32r format)."""
    v = x.astype(np.float32).view(np.uint32).astype(np.uint64)
    lsb = (v >> 12) & 1
    v = v + 0x7FF + lsb
    v = (v & np.uint64(0xFFFFF000)).astype(np.uint32)
    return v.view(np.float32)


def build_nc(n_reps: int = 1, gelu: bool = True, phase_a: bool = True,
             phase_b: bool = True):
    act_fn = AF.Gelu_apprx_tanh if gelu else AF.Copy
    nc = bacc.Bacc("TRN2", target_bir_lowering=False, debug=False, num_devices=N_CORES)
    vb = nc.dram_tensor("vb", [N1, TPC, D], f32, kind="ExternalInput").ap()
    w1 = nc.dram_tensor("w1", [D, F], f32r, kind="ExternalInput").ap()
    w2 = nc.dram_tensor("w2", [F, D], f32r, kind="ExternalInput").ap()
    pjw = nc.dram_tensor("pjw", [D], f32, kind="ExternalInput").ap()
    nsw = nc.dram_tensor("nsw", [D], f32, kind="ExternalInput").ap()
    h_out = nc.dram_tensor("h_out", [TPC, D], f32, kind="ExternalOutput").ap()
    np_out = nc.dram_tensor("np_out", [TPC, D], f32, kind="ExternalOutput").ap()

    w1g = w1.rearrange("(kc p) f -> p kc f", p=P)   # [128, 16, 8192]
    h_out_t = h_out.rearrange("(tt p) d -> tt p d", p=P)

    with tile.TileContext(nc) as tc, ExitStack() as ctx:
        outer = ctx.enter_context(tc.tile_pool(name="outer", bufs=1))
        pw_b = outer.tile([P, D], f32)                                  # bcast proj_w*norm_scale
        hT = [outer.tile([P, TPC], f32r, name=f"hT{k}") for k in range(DC)]

        for _rep in range(n_reps):
            # ---------------- Phase A: block attention -> h, hT ----------------
            if not phase_a:
                zp = ctx.enter_context(tc.tile_pool(name="zp", bufs=1))
                zt = zp.tile([P, TPC], f32)
                nc.vector.memset(zt, 0.001)
                for k in range(DC):
                    nc.scalar.activation(hT[k][:], zt[:], AF.Copy)
            if phase_a:
              with ExitStack() as ctxA:
                vpool = ctxA.enter_context(tc.tile_pool(name="vpool", bufs=10))
                spool = ctxA.enter_context(tc.tile_pool(name="spool", bufs=2))
                sqps = ctxA.enter_context(tc.tile_pool(name="sqps", bufs=1))
                small = ctxA.enter_context(tc.tile_pool(name="small", bufs=3))
                hpool = ctxA.enter_context(tc.tile_pool(name="hpool", bufs=2))
                psumT = ctxA.enter_context(tc.tile_pool(name="psumT", bufs=4, space="PSUM"))
                consts = ctxA.enter_context(tc.tile_pool(name="consts", bufs=1))

                ident = consts.tile([P, P], f32)
                make_identity(nc, ident)
                eps_t = consts.tile([P, 1], f32)
                nc.vector.memset(eps_t, EPS)
                nb_t = consts.tile([P, D], f32)
                nsw_b = bass.AP(tensor=nsw.tensor, offset=nsw.offset,
                                ap=[[0, P], *nsw.ap])
                nc.gpsimd.dma_start(out=nb_t, in_=nsw_b)
                pj_t = consts.tile([P, D], f32)
                pjw_b = bass.AP(tensor=pjw.tensor, offset=pjw.offset,
                                ap=[[0, P], *pjw.ap])
                nc.gpsimd.dma_start(out=pj_t, in_=pjw_b)
                nc.vector.tensor_mul(pw_b[:], nb_t[:], pj_t[:])

                for tt in range(TT):
                    ss9 = small.tile([P, N1], f32, name="ss9")
                    dp9 = small.tile([P, N1], f32, name="dp9")
                    vts = []
                    for n in range(N1):
                        v = vpool.tile([P, D], f32, name="vt")
                        nc.sync.dma_start(out=v, in_=vb[n, tt * P:(tt + 1) * P, :])
                        vts.append(v)
                        sq = sqps.tile([P, D], f32, name="sq")
                        nc.scalar.activation(sq[:], v[:], AF.Square,
                                             accum_out=ss9[:, n:n + 1])
                        dsc = spool.tile([P, D], f32, name="dsc")
                        nc.vector.scalar_tensor_tensor(
                            out=dsc[:], in0=v[:], scalar=1.0, in1=pw_b[:],
                            op0=ALU.mult, op1=ALU.mult, accum_out=dp9[:, n:n + 1])
                    rms9 = small.tile([P, N1], f32, name="rms9")
                    nc.scalar.activation(rms9[:], ss9[:], AF.Sqrt,
                                         bias=eps_t[:], scale=1.0 / D)
                    inv9 = small.tile([P, N1], f32, name="inv9")
                    nc.vector.reciprocal(inv9[:], rms9[:])
                    lg9 = small.tile([P, N1], f32, name="lg9")
                    nc.vector.tensor_mul(lg9[:], dp9[:], inv9[:])
                    mx1 = small.tile([P, 1], f32, name="mx1")
                    nc.vector.tensor_reduce(mx1[:], lg9[:], axis=mybir.AxisListType.X,
                                            op=ALU.max)
                    nc.vector.tensor_scalar_sub(lg9[:], lg9[:], mx1[:])
                    e9 = small.tile([P, N1], f32, name="e9")
                    se1 = small.tile([P, 1], f32, name="se1")
                    nc.scalar.activation(e9[:], lg9[:], AF.Exp, accum_out=se1[:])
                    invs = small.tile([P, 1], f32, name="invs")
                    nc.vector.reciprocal(invs[:], se1[:])
                    al9 = small.tile([P, N1], f32, name="al9")
                    nc.vector.tensor_scalar_mul(al9[:], e9[:], invs[:])

                    h_t = hpool.tile([P, D], f32, name="ht")
                    nc.vector.tensor_scalar_mul(h_t[:], vts[0][:], al9[:, 0:1])
                    for n in range(1, N1):
                        nc.vector.scalar_tensor_tensor(
                            out=h_t[:], in0=vts[n][:], scalar=al9[:, n:n + 1],
                            in1=h_t[:], op0=ALU.mult, op1=ALU.add)
                    nc.sync.dma_start(out=h_out_t[tt], in_=h_t[:])
                    for k in range(DC):
                        pst = psumT.tile([P, P], f32, name="pst")
                        nc.tensor.transpose(pst[:], h_t[:, k * P:(k + 1) * P], ident[:])
                        nc.scalar.activation(hT[k][:, tt * P:(tt + 1) * P], pst[:],
                                             AF.Copy)

            # ---------------- Phase B: FFN (f32r) + residual ----------------
            if phase_b:
              with ExitStack() as ctxB:
                w1p = ctxB.enter_context(tc.tile_pool(name="w1p", bufs=3))
                w2p = ctxB.enter_context(tc.tile_pool(name="w2p", bufs=4))
                actp = ctxB.enter_context(tc.tile_pool(name="actp", bufs=FH + 4))
                oap = ctxB.enter_context(tc.tile_pool(name="oap", bufs=1))
                evp = ctxB.enter_context(tc.tile_pool(name="evp", bufs=4))
                ptp = ctxB.enter_context(tc.tile_pool(name="ptp", bufs=4))
                ps1p = ctxB.enter_context(tc.tile_pool(name="ps1p", bufs=2, space="PSUM"))
                ps2p = ctxB.enter_context(tc.tile_pool(name="ps2p", bufs=4, space="PSUM"))

                out_acc = [oap.tile([P, D], f32, name=f"oa{m}") for m in range(TT)]

                for half in range(2):
                    act_tiles = []
                    for fcl in range(FH):
                        gfc = half * FH + fcl
                        w1t = w1p.tile([P, DC, P], f32r, name="w1t")
                        nc.sync.dma_start(out=w1t, in_=w1g[:, :, gfc * P:(gfc + 1) * P])
                        ps1 = ps1p.tile([P, TPC], f32, name="ps1")
                        for k in range(DC):
                            nc.tensor.matmul(ps1[:], lhsT=w1t[:, k, :], rhs=hT[k][:],
                                             start=(k == 0), stop=(k == DC - 1))
                        a_t = actp.tile([P, TPC], f32r, name="act")
                        nc.scalar.activation(a_t[:], ps1[:], act_fn)
                        act_tiles.append(a_t)

                    for q in range(NQ):
                        ps2 = [ps2p.tile([P, 512], f32, name="ps2") for _ in range(TT)]
                        for fcl in range(FH):
                            gfc = half * FH + fcl
                            w2t = w2p.tile([P, 512], f32r, name="w2t")
                            nc.sync.dma_start(
                                out=w2t,
                                in_=w2[gfc * P:(gfc + 1) * P, q * 512:(q + 1) * 512])
                            for m in range(TT):
                                nc.tensor.matmul(
                                    ps2[m][:],
                                    lhsT=act_tiles[fcl][:, m * P:(m + 1) * P],
                                    rhs=w2t[:],
                                    start=(fcl == 0), stop=(fcl == FH - 1))
                        for m in range(TT):
                            if half == 0:
                                nc.vector.tensor_copy(
                                    out_acc[m][:, q * 512:(q + 1) * 512], ps2[m][:])
                            else:
                                ev = evp.tile([P, 512], f32, name="ev")
                                nc.vector.tensor_add(
                                    ev[:], ps2[m][:],
                                    out_acc[m][:, q * 512:(q + 1) * 512])
                                pt = ptp.tile([P, 512], f32, name="pt")
                                nc.sync.dma_start(
                                    out=pt,
                                    in_=vb[NB, m * P:(m + 1) * P, q * 512:(q + 1) * 512])
                                nc.vector.tensor_add(ev[:], ev[:], pt[:])
                                nc.sync.dma_start(
                                    out=np_out[m * P:(m + 1) * P, q * 512:(q + 1) * 512],
                                    in_=ev[:])

    nc.compile()
    return nc


_NC = None


def _get_nc():
    global _NC
    if _NC is None:
        _NC = build_nc()
    return _NC


def kernel(blocks, partial_block, proj_w, norm_scale, ffn_w1, ffn_w2):
    blocks = np.ascontiguousarray(np.asarray(blocks, dtype=np.float32)).reshape(NB, TOK, D)
    pb = np.ascontiguousarray(np.asarray(partial_block, dtype=np.float32)).reshape(TOK, D)
    w1r = round_f32r(np.asarray(ffn_w1, dtype=np.float32))
    w2r = round_f32r(np.asarray(ffn_w2, dtype=np.float32))
    pjw = np.ascontiguousarray(np.asarray(proj_w, dtype=np.float32))
    nsw = np.ascontiguousarray(np.asarray(norm_scale, dtype=np.float32))

    in_maps = []
    for c in range(N_CORES):
        sl = slice(c * TPC, (c + 1) * TPC)
        vbc = np.concatenate([blocks[:, sl], pb[None, sl]], axis=0)
        in_maps.append({"vb": vbc, "w1": w1r, "w2": w2r, "pjw": pjw, "nsw": nsw})

    nc = _get_nc()
    res = run_bass_kernel_spmd(nc, in_maps, list(range(N_CORES)))
    h = np.concatenate([r["h_out"] for r in res.results], axis=0).reshape(B, T, D)
    npar = np.concatenate([r["np_out"] for r in res.results], axis=0).reshape(B, T, D)
    return h, npar
